# Initial kernel scaffold
#
"""GQA attention (S=2048, D=4096, H=32, G=8, DH=128) on 8 trn2 cores.

Sharding: core i owns query heads [4i, 4i+4) and KV group i (column shards
of Wq/Wk/Wv). After attention each core holds a normalized context slice
ctxT_i [512, 2048] (feature-major); chunked AllGathers (one per 512-query
chunk, overlapped with compute) assemble the full ctxT [4096, 2048] and each
core then computes its 512-column shard of the output projection. The host
concatenates the 8 column shards.

All activations are kept feature-major ([feature, seq]) so the attention
pipeline needs no Q/K/score transposes:
  qT_h = Wq_h^T @ x^T           (PE, accumulate over D tiles)
  RoPE applied via a signed half-swap permutation matmul + DVE muls
  s[t,q] block = kT_tile.T @ qT chunk      (scoresT layout)
  p    = exp(s/sqrt(DH) - 4)    (ACT; constant bias keeps p in fp16 range,
                                 softmax-invariant; no max subtraction
                                 needed: |s/sqrt(DH)| <= ~14)
  den  = ones^T @ sum_t p       (DVE partial sums + one ones-matmul)
  ctxT = v_block.T @ p          (PE accumulate; v transposed at proj time)
  out  = ctx_tile.T @ Wo_shard  (PE, per-chunk after its AllGather)
Matmul operands are fp16 (1 cycle/row on the PE, fp32 PSUM accumulation;
fp16's 11-bit mantissa keeps the end-to-end error ~1e-3). Phase C is
emitted interleaved with phase B so the tensor engine stays dense while
the scalar engine works through the exps.
"""

import math
import sys

if "/opt/trn_rl_repo" not in sys.path:
    sys.path.insert(0, "/opt/trn_rl_repo")

import numpy as np

S, D, H, G, DH = 2048, 4096, 32, 8, 128
N_CORES = 8
HPC = H // N_CORES          # query heads per core (4)
FPC = HPC * DH              # context features per core (512)
QC = 512                    # query chunk (matmul free dim)
NQC = S // QC               # 4
TB = 128                    # key block
NTB = S // TB               # 16
NKT = D // 128              # contraction tiles over D (32)
NJ = QC // TB               # key blocks per query chunk (4)
INV_SQRT_DH = 1.0 / math.sqrt(DH)
EXP_BIAS = -4.0             # keeps exp() outputs inside fp16 range
NEG_BIAS = -1.0e30

_CACHE = {}


def _build_program():
    import concourse.mybir as mybir
    import concourse.tile as tile
    from concourse import bacc

    f32 = mybir.dt.float32
    f16 = mybir.dt.float16
    EXP = mybir.ActivationFunctionType.Exp

    nc = bacc.Bacc("TRN2", target_bir_lowering=False, debug=False,
                   num_devices=N_CORES)

    xT = nc.dram_tensor("xT", [D, S], f16, kind="ExternalInput")
    wq_d = nc.dram_tensor("wq", [D, FPC], f16, kind="ExternalInput")
    wk_d = nc.dram_tensor("wk", [D, DH], f16, kind="ExternalInput")
    wv_d = nc.dram_tensor("wv", [D, DH], f16, kind="ExternalInput")
    wo_d = nc.dram_tensor("wo", [D, FPC], f16, kind="ExternalInput")
    cosT_d = nc.dram_tensor("cosT", [DH, S], f32, kind="ExternalInput")
    sinT_d = nc.dram_tensor("sinT", [DH, S], f32, kind="ExternalInput")
    maskb_d = nc.dram_tensor("maskb", [NJ, TB, QC], f32, kind="ExternalInput")
    ident_d = nc.dram_tensor("ident", [TB, TB], f16, kind="ExternalInput")
    rotm_d = nc.dram_tensor("rotm", [DH, DH], f16, kind="ExternalInput")
    onesc_d = nc.dram_tensor("onesc", [TB, 1], f16, kind="ExternalInput")
    onesr_d = nc.dram_tensor("onesr", [1, DH], f16, kind="ExternalInput")
    out_d = nc.dram_tensor("out", [S, FPC], f32, kind="ExternalOutput")

    with tile.TileContext(nc) as tc:
        with tc.tile_pool(name="dram", bufs=1, space="DRAM") as dram:
            ctx_loc = [dram.tile([FPC, QC], f16, name=f"ctx_loc{qc}",
                                 tag=f"cl{qc}") for qc in range(NQC)]
            ctx_all = [dram.tile([N_CORES * FPC, QC], f16, name=f"ctx_all{qc}",
                                 tag=f"ca{qc}", addr_space="Shared")
                       for qc in range(NQC)]

            with tc.tile_pool(name="res", bufs=1) as res:
                # --- small resident constants ---
                ident_sb = res.tile([TB, TB], f16, tag="ident", name="ident_sb")
                nc.sync.dma_start(out=ident_sb[:], in_=ident_d[:])
                rotm_sb = res.tile([DH, DH], f16, tag="rotm", name="rotm_sb")
                nc.sync.dma_start(out=rotm_sb[:], in_=rotm_d[:])
                onesc_sb = res.tile([TB, 1], f16, tag="onesc", name="onesc_sb")
                nc.sync.dma_start(out=onesc_sb[:], in_=onesc_d[:])
                onesr_sb = res.tile([1, DH], f16, tag="onesr", name="onesr_sb")
                nc.sync.dma_start(out=onesr_sb[:], in_=onesr_d[:])
                ebias_sb = res.tile([128, 1], f32, tag="ebias", name="ebias_sb")
                nc.vector.memset(ebias_sb[:], EXP_BIAS)
                masks_sb = res.tile([TB, NJ, QC], f32, tag="masks", name="masks_sb")
                for j in range(NJ):
                    nc.sync.dma_start(out=masks_sb[:, j, :], in_=maskb_d[j])

                # --- persistent activations ---
                qT_sb = []
                for h in range(HPC):
                    qh = res.tile([128, S], f16, tag=f"qT{h}", name=f"qT{h}_sb")
                    qT_sb.append(qh)
                kT_sb = res.tile([128, S], f16, tag="kT", name="kT_sb")
                v_sb = res.tile([128, NTB, TB], f16, tag="v", name="v_sb")

                # big weight slot: wq during phase A, wo afterwards
                wq_sb = res.tile([128, NKT, FPC], f16, tag="bigw", name="wq_sb")

                # ---- phase A ----
                with tc.tile_pool(name="pA", bufs=1) as pA, \
                     tc.tile_pool(name="psA", bufs=1, space="PSUM") as psA:
                    wk_sb = pA.tile([128, NKT, DH], f16, tag="wk", name="wk_sb")
                    wv_sb = pA.tile([128, NKT, DH], f16, tag="wv", name="wv_sb")

                    def rope(src_ps, dst_ap, cos_c, sin_c):
                        qc_sb = pA.tile([128, QC], f16, tag="ropecp", bufs=2,
                                        name="qc_sb")
                        nc.scalar.copy(qc_sb[:], src_ps[:])
                        rot_ps = psA.tile([128, QC], f32, tag="rot", bufs=1,
                                          name="rot_ps")
                        nc.tensor.matmul(rot_ps[:], rotm_sb[:], qc_sb[:],
                                         start=True, stop=True)
                        t1 = pA.tile([128, QC], f32, tag="ropet1", bufs=2,
                                     name="t1")
                        nc.vector.tensor_mul(t1[:], rot_ps[:], sin_c[:])
                        nc.vector.tensor_mul(dst_ap, qc_sb[:], cos_c[:])
                        nc.vector.tensor_add(dst_ap, dst_ap, t1[:])

                    for c in range(NQC):
                        csl = slice(c * QC, (c + 1) * QC)
                        cos_c = pA.tile([DH, QC], f32, tag="cosc", bufs=2,
                                        name="cos_c")
                        nc.sync.dma_start(out=cos_c[:], in_=cosT_d[:, csl])
                        sin_c = pA.tile([DH, QC], f32, tag="sinc", bufs=2,
                                        name="sin_c")
                        nc.sync.dma_start(out=sin_c[:], in_=sinT_d[:, csl])
                        q_ps = []
                        for h in range(HPC):
                            qp = psA.tile([128, QC], f32, tag=f"pq{h}", bufs=1,
                                          name=f"q_ps{h}")
                            q_ps.append(qp)
                        k_ps = psA.tile([128, QC], f32, tag="pk", bufs=1,
                                        name="k_ps")
                        vT_ps = psA.tile([128, QC], f32, tag="pv", bufs=1,
                                         name="vT_ps")
                        for kt in range(NKT):
                            ksl = slice(kt * 128, (kt + 1) * 128)
                            xt = pA.tile([128, QC], f16, tag="xt", bufs=4,
                                         name="xt")
                            nc.sync.dma_start(out=xt[:], in_=xT[ksl, csl])
                            if c == 0:
                                # interleave weight loads with the x stream so
                                # compute starts immediately
                                nc.sync.dma_start(out=wq_sb[:, kt, :],
                                                  in_=wq_d[ksl, :])
                                nc.sync.dma_start(out=wk_sb[:, kt, :],
                                                  in_=wk_d[ksl, :])
                                nc.sync.dma_start(out=wv_sb[:, kt, :],
                                                  in_=wv_d[ksl, :])
                            st, sp = kt == 0, kt == NKT - 1
                            for h in range(HPC):
                                nc.tensor.matmul(
                                    q_ps[h][:],
                                    wq_sb[:, kt, h * DH:(h + 1) * DH],
                                    xt[:], start=st, stop=sp)
                            nc.tensor.matmul(k_ps[:], wk_sb[:, kt, :], xt[:],
                                             start=st, stop=sp)
                            nc.tensor.matmul(vT_ps[:], wv_sb[:, kt, :], xt[:],
                                             start=st, stop=sp)
                        for h in range(HPC):
                            rope(q_ps[h], qT_sb[h][:, csl], cos_c, sin_c)
                        rope(k_ps, kT_sb[:, csl], cos_c, sin_c)
                        vts = pA.tile([128, QC], f16, tag="vts", bufs=2,
                                      name="vts")
                        nc.scalar.copy(vts[:], vT_ps[:])
                        for sb in range(NJ):
                            tr_ps = psA.tile([TB, TB], f16, tag="tr", bufs=1,
                                             name="tr_ps")
                            nc.tensor.transpose(tr_ps[:],
                                                vts[:, sb * TB:(sb + 1) * TB],
                                                ident_sb[:])
                            nc.scalar.copy(v_sb[:, c * NJ + sb, :], tr_ps[:])

                # wo reuses the wq slot; DMAs overlap phase B compute
                wo_sb = res.tile([128, NKT, FPC], f16, tag="bigw", name="wo_sb")
                for kt in range(NKT):
                    nc.sync.dma_start(out=wo_sb[:, kt, :],
                                      in_=wo_d[kt * 128:(kt + 1) * 128, :])

                # ---- phases B (attention) + chunked allgather + C, fused ----
                with tc.tile_pool(name="pB", bufs=1) as pB, \
                     tc.tile_pool(name="psB", bufs=1, space="PSUM") as psB, \
                     tc.tile_pool(name="pC", bufs=1) as pC, \
                     tc.tile_pool(name="psC", bufs=1, space="PSUM") as psC:

                    def phase_b(qcn):
                        qsl = slice(qcn * QC, (qcn + 1) * QC)
                        ntb = (qcn + 1) * NJ
                        ctx_ps = [psB.tile([128, QC], f32, tag=f"ctx{h}",
                                           bufs=1, name=f"ctx_ps{h}")
                                  for h in range(HPC)]
                        den_sb = [pB.tile([128, QC], f32, tag=f"den{h}",
                                          bufs=1, name=f"den_sb{h}")
                                  for h in range(HPC)]
                        den_r = [None] * HPC
                        for tb in range(ntb):
                            j = tb - qcn * NJ
                            p_list = []
                            for h in range(HPC):
                                s_ps = psB.tile([128, QC], f32, tag="s",
                                                bufs=2, name="s_ps")
                                nc.tensor.matmul(
                                    s_ps[:], kT_sb[:, tb * TB:(tb + 1) * TB],
                                    qT_sb[h][:, qsl], start=True, stop=True)
                                if j >= 0:
                                    nc.vector.tensor_add(s_ps[:], s_ps[:],
                                                         masks_sb[:, j, :])
                                p_sb = pB.tile([128, QC], f16, tag="p",
                                               bufs=8, name="p_sb")
                                nc.scalar.activation(p_sb[:], s_ps[:], EXP,
                                                     bias=ebias_sb[:],
                                                     scale=INV_SQRT_DH)
                                p_list.append(p_sb)
                                if tb == 0:
                                    nc.vector.tensor_copy(den_sb[h][:], p_sb[:])
                                elif tb == ntb - 1:
                                    dr = pB.tile([128, QC], f16, tag="denr_sb",
                                                 bufs=2, name="den_r")
                                    nc.vector.tensor_add(dr[:], den_sb[h][:],
                                                         p_sb[:])
                                    den_r[h] = dr
                                else:
                                    nc.vector.tensor_add(den_sb[h][:],
                                                         den_sb[h][:], p_sb[:])
                            for h in range(HPC):
                                nc.tensor.matmul(ctx_ps[h][:], v_sb[:, tb, :],
                                                 p_list[h][:],
                                                 start=(tb == 0),
                                                 stop=(tb == ntb - 1))
                        for h in range(HPC):
                            aux1 = psB.tile([128, QC], f32, tag="aux", bufs=1,
                                            name="aux1")
                            nc.tensor.matmul(aux1[:1, :], onesc_sb[:],
                                             den_r[h][:], start=True, stop=True)
                            recip_sb = pB.tile([1, QC], f16, tag="recip",
                                               bufs=2, name="recip_sb")
                            with nc.allow_low_precision(reason="softmax denom"):
                                nc.vector.reciprocal(recip_sb[:], aux1[:1, :])
                            aux2 = psB.tile([128, QC], f32, tag="aux", bufs=1,
                                            name="aux2")
                            nc.tensor.matmul(aux2[:], onesr_sb[:], recip_sb[:],
                                             start=True, stop=True)
                            ctmp = pB.tile([128, QC], f32, tag="ctmp",
                                           bufs=2, name="ctmp")
                            nc.scalar.copy(ctmp[:], ctx_ps[h][:])
                            ctx_sb = pB.tile([128, QC], f16, tag="ctxsb",
                                             bufs=4, name="ctx_sb")
                            nc.vector.tensor_mul(ctx_sb[:], ctmp[:], aux2[:])
                            nc.sync.dma_start(
                                out=ctx_loc[qcn][h * DH:(h + 1) * DH, :],
                                in_=ctx_sb[:])
                        nc.gpsimd.collective_compute(
                            "AllGather", mybir.AluOpType.bypass,
                            replica_groups=[list(range(N_CORES))],
                            ins=[ctx_loc[qcn].opt()],
                            outs=[ctx_all[qcn].opt()])

                    def phase_c(qcn):
                        for qb in range(NJ):
                            bsl = slice(qb * TB, (qb + 1) * TB)
                            o_ps = psC.tile([TB, FPC], f32, tag="o",
                                            bufs=1, name="o_ps")
                            for kt in range(NKT):
                                ct = pC.tile([128, TB], f16, tag="ct",
                                             bufs=6, name="ct")
                                nc.sync.dma_start(
                                    out=ct[:],
                                    in_=ctx_all[qcn][kt * 128:(kt + 1) * 128,
                                                     bsl])
                                nc.tensor.matmul(o_ps[:], ct[:],
                                                 wo_sb[:, kt, :],
                                                 start=(kt == 0),
                                                 stop=(kt == NKT - 1))
                            o_sb = pC.tile([TB, FPC], f32, tag="osb",
                                           bufs=3, name="o_sb")
                            nc.scalar.copy(o_sb[:], o_ps[:])
                            qrow = qcn * QC + qb * TB
                            nc.sync.dma_start(out=out_d[qrow:qrow + TB, :],
                                              in_=o_sb[:])

                    phase_b(0)
                    for qcn in range(1, NQC):
                        phase_b(qcn)
                        phase_c(qcn - 1)
                    phase_c(NQC - 1)
    nc.compile()
    return nc


def _host_consts():
    ident = np.eye(TB, dtype=np.float16)
    rotm = np.zeros((DH, DH), dtype=np.float16)
    half = DH // 2
    for d in range(half):
        rotm[d + half, d] = -1.0   # out[d] = -q[d+half]
        rotm[d, d + half] = 1.0    # out[d+half] = q[d]
    onesc = np.ones((TB, 1), dtype=np.float16)
    onesr = np.ones((1, DH), dtype=np.float16)
    maskb = np.zeros((NJ, TB, QC), dtype=np.float32)
    tloc = np.arange(TB)[:, None]
    qloc = np.arange(QC)[None, :]
    for j in range(NJ):
        maskb[j] = np.where(tloc + TB * j <= qloc, 0.0, NEG_BIAS)
    return ident, rotm, onesc, onesr, maskb


def kernel(x, mask, cos, sin, Wq, Wk, Wv, Wo):
    from concourse.bass_utils import run_bass_kernel_spmd

    if "nc" not in _CACHE:
        _CACHE["nc"] = _build_program()
    nc = _CACHE["nc"]

    x = np.asarray(x, dtype=np.float32)
    cos = np.asarray(cos, dtype=np.float32)
    sin = np.asarray(sin, dtype=np.float32)
    Wq = np.asarray(Wq, dtype=np.float32)
    Wk = np.asarray(Wk, dtype=np.float32)
    Wv = np.asarray(Wv, dtype=np.float32)
    Wo = np.asarray(Wo, dtype=np.float32)

    xT = np.ascontiguousarray(x[0].T).astype(np.float16)   # [D, S]
    cosT = np.ascontiguousarray(cos.T)                     # [DH, S]
    sinT = np.ascontiguousarray(sin.T)
    ident, rotm, onesc, onesr, maskb = _host_consts()

    in_maps = []
    for i in range(N_CORES):
        in_maps.append({
            "xT": xT,
            "wq": np.ascontiguousarray(Wq[:, i * FPC:(i + 1) * FPC]).astype(np.float16),
            "wk": np.ascontiguousarray(Wk[:, i * DH:(i + 1) * DH]).astype(np.float16),
            "wv": np.ascontiguousarray(Wv[:, i * DH:(i + 1) * DH]).astype(np.float16),
            "wo": np.ascontiguousarray(Wo[:, i * FPC:(i + 1) * FPC]).astype(np.float16),
            "cosT": cosT,
            "sinT": sinT,
            "maskb": maskb,
            "ident": ident,
            "rotm": rotm,
            "onesc": onesc,
            "onesr": onesr,
        })

    import os
    trace = bool(os.environ.get("BASS_TRACE"))
    res = run_bass_kernel_spmd(nc, in_maps, list(range(N_CORES)), trace=trace)
    _CACHE["last_exec_time_ns"] = res.exec_time_ns

    out = np.concatenate([res.results[i]["out"] for i in range(N_CORES)], axis=1)
    return out[None]



# revision 1
# speedup vs baseline: 1.0003x; 1.0003x over previous
"""GQA attention (S=2048, D=4096, H=32, G=8, DH=128) on 8 trn2 cores.

Sharding: core i owns query heads [4i, 4i+4) and KV group i (column shards
of Wq/Wk/Wv). After attention each core holds a normalized context slice
ctxT_i [512, 2048] (feature-major); chunked AllGathers (one per 512-query
chunk, overlapped with compute) assemble the full ctxT [4096, 2048] and each
core then computes its 512-column shard of the output projection. The host
concatenates the 8 column shards.

All activations are kept feature-major ([feature, seq]) so the attention
pipeline needs no Q/K/score transposes:
  qT_h = Wq_h^T @ x^T           (PE, accumulate over D tiles)
  RoPE applied via a signed half-swap permutation matmul + DVE muls
  s[t,q] block = kT_tile.T @ qT chunk      (scoresT layout)
  p    = exp(s/sqrt(DH) - 4)    (ACT; constant bias keeps p in fp16 range,
                                 softmax-invariant; no max subtraction
                                 needed: |s/sqrt(DH)| <= ~14)
  den  = ones^T @ sum_t p       (DVE partial sums + one ones-matmul)
  ctxT = v_block.T @ p          (PE accumulate; v transposed at proj time)
  out  = ctx_tile.T @ Wo_shard  (PE, per-chunk after its AllGather)
Matmul operands are fp16 (1 cycle/row on the PE, fp32 PSUM accumulation;
fp16's 11-bit mantissa keeps the end-to-end error ~1e-3). Phase C is
emitted interleaved with phase B so the tensor engine stays dense while
the scalar engine works through the exps.
"""

import math
import sys

if "/opt/trn_rl_repo" not in sys.path:
    sys.path.insert(0, "/opt/trn_rl_repo")

import numpy as np

S, D, H, G, DH = 2048, 4096, 32, 8, 128
N_CORES = 8
HPC = H // N_CORES          # query heads per core (4)
FPC = HPC * DH              # context features per core (512)
QC = 512                    # query chunk (matmul free dim)
NQC = S // QC               # 4
TB = 128                    # key block
NTB = S // TB               # 16
NKT = D // 128              # contraction tiles over D (32)
NJ = QC // TB               # key blocks per query chunk (4)
INV_SQRT_DH = 1.0 / math.sqrt(DH)
EXP_BIAS = -4.0             # keeps exp() outputs inside fp16 range
NEG_BIAS = -1.0e30

_CACHE = {}


def _build_program():
    import concourse.mybir as mybir
    import concourse.tile as tile
    from concourse import bacc

    f32 = mybir.dt.float32
    f16 = mybir.dt.float16
    EXP = mybir.ActivationFunctionType.Exp

    nc = bacc.Bacc("TRN2", target_bir_lowering=False, debug=False,
                   num_devices=N_CORES)

    xT = nc.dram_tensor("xT", [D, S], f16, kind="ExternalInput")
    wq_d = nc.dram_tensor("wq", [D, FPC], f16, kind="ExternalInput")
    wk_d = nc.dram_tensor("wk", [D, DH], f16, kind="ExternalInput")
    wv_d = nc.dram_tensor("wv", [D, DH], f16, kind="ExternalInput")
    wo_d = nc.dram_tensor("wo", [D, FPC], f16, kind="ExternalInput")
    cosT_d = nc.dram_tensor("cosT", [DH, S], f32, kind="ExternalInput")
    sinT_d = nc.dram_tensor("sinT", [DH, S], f32, kind="ExternalInput")
    maskb_d = nc.dram_tensor("maskb", [NJ, TB, QC], f32, kind="ExternalInput")
    ident_d = nc.dram_tensor("ident", [TB, TB], f16, kind="ExternalInput")
    rotm_d = nc.dram_tensor("rotm", [DH, DH], f16, kind="ExternalInput")
    onesc_d = nc.dram_tensor("onesc", [TB, 1], f16, kind="ExternalInput")
    onesr_d = nc.dram_tensor("onesr", [1, DH], f16, kind="ExternalInput")
    out_d = nc.dram_tensor("out", [S, FPC], f32, kind="ExternalOutput")

    with tile.TileContext(nc) as tc:
        with tc.tile_pool(name="dram", bufs=1, space="DRAM") as dram:
            ctx_loc = [dram.tile([FPC, QC], f16, name=f"ctx_loc{qc}",
                                 tag=f"cl{qc}") for qc in range(NQC)]
            ctx_all = [dram.tile([N_CORES * FPC, QC], f16, name=f"ctx_all{qc}",
                                 tag=f"ca{qc}", addr_space="Shared")
                       for qc in range(NQC)]

            with tc.tile_pool(name="res", bufs=1) as res:
                # --- small resident constants ---
                ident_sb = res.tile([TB, TB], f16, tag="ident", name="ident_sb")
                nc.sync.dma_start(out=ident_sb[:], in_=ident_d[:])
                rotm_sb = res.tile([DH, DH], f16, tag="rotm", name="rotm_sb")
                nc.sync.dma_start(out=rotm_sb[:], in_=rotm_d[:])
                onesc_sb = res.tile([TB, 1], f16, tag="onesc", name="onesc_sb")
                nc.sync.dma_start(out=onesc_sb[:], in_=onesc_d[:])
                onesr_sb = res.tile([1, DH], f16, tag="onesr", name="onesr_sb")
                nc.sync.dma_start(out=onesr_sb[:], in_=onesr_d[:])
                ebias_sb = res.tile([128, 1], f32, tag="ebias", name="ebias_sb")
                nc.vector.memset(ebias_sb[:], EXP_BIAS)
                masks_sb = res.tile([TB, NJ, QC], f32, tag="masks", name="masks_sb")
                for j in range(NJ):
                    nc.sync.dma_start(out=masks_sb[:, j, :], in_=maskb_d[j])

                # --- persistent activations ---
                qT_sb = []
                for h in range(HPC):
                    qh = res.tile([128, S], f16, tag=f"qT{h}", name=f"qT{h}_sb")
                    qT_sb.append(qh)
                kT_sb = res.tile([128, S], f16, tag="kT", name="kT_sb")
                v_sb = res.tile([128, NTB, TB], f16, tag="v", name="v_sb")

                # big weight slot: wq during phase A, wo afterwards
                wq_sb = res.tile([128, NKT, FPC], f16, tag="bigw", name="wq_sb")

                # ---- phase A ----
                with tc.tile_pool(name="pA", bufs=1) as pA, \
                     tc.tile_pool(name="psA", bufs=1, space="PSUM") as psA:
                    wk_sb = pA.tile([128, NKT, DH], f16, tag="wk", name="wk_sb")
                    wv_sb = pA.tile([128, NKT, DH], f16, tag="wv", name="wv_sb")

                    def rope(src_ps, dst_ap, cos_c, sin_c):
                        qc_sb = pA.tile([128, QC], f16, tag="ropecp", bufs=2,
                                        name="qc_sb")
                        nc.scalar.copy(qc_sb[:], src_ps[:])
                        rot_ps = psA.tile([128, QC], f32, tag="rot", bufs=1,
                                          name="rot_ps")
                        nc.tensor.matmul(rot_ps[:], rotm_sb[:], qc_sb[:],
                                         start=True, stop=True)
                        t1 = pA.tile([128, QC], f32, tag="ropet1", bufs=2,
                                     name="t1")
                        nc.vector.tensor_mul(t1[:], rot_ps[:], sin_c[:])
                        nc.vector.tensor_mul(dst_ap, qc_sb[:], cos_c[:])
                        nc.vector.tensor_add(dst_ap, dst_ap, t1[:])

                    for c in range(NQC):
                        csl = slice(c * QC, (c + 1) * QC)
                        cos_c = pA.tile([DH, QC], f32, tag="cosc", bufs=2,
                                        name="cos_c")
                        nc.sync.dma_start(out=cos_c[:], in_=cosT_d[:, csl])
                        sin_c = pA.tile([DH, QC], f32, tag="sinc", bufs=2,
                                        name="sin_c")
                        nc.sync.dma_start(out=sin_c[:], in_=sinT_d[:, csl])
                        q_ps = []
                        for h in range(HPC):
                            qp = psA.tile([128, QC], f32, tag=f"pq{h}", bufs=1,
                                          name=f"q_ps{h}")
                            q_ps.append(qp)
                        k_ps = psA.tile([128, QC], f32, tag="pk", bufs=1,
                                        name="k_ps")
                        vT_ps = psA.tile([128, QC], f32, tag="pv", bufs=1,
                                         name="vT_ps")
                        for kt in range(NKT):
                            ksl = slice(kt * 128, (kt + 1) * 128)
                            xt = pA.tile([128, QC], f16, tag="xt", bufs=4,
                                         name="xt")
                            nc.sync.dma_start(out=xt[:], in_=xT[ksl, csl])
                            if c == 0:
                                # interleave weight loads with the x stream so
                                # compute starts immediately
                                nc.sync.dma_start(out=wq_sb[:, kt, :],
                                                  in_=wq_d[ksl, :])
                                nc.sync.dma_start(out=wk_sb[:, kt, :],
                                                  in_=wk_d[ksl, :])
                                nc.sync.dma_start(out=wv_sb[:, kt, :],
                                                  in_=wv_d[ksl, :])
                            st, sp = kt == 0, kt == NKT - 1
                            for h in range(HPC):
                                nc.tensor.matmul(
                                    q_ps[h][:],
                                    wq_sb[:, kt, h * DH:(h + 1) * DH],
                                    xt[:], start=st, stop=sp)
                            nc.tensor.matmul(k_ps[:], wk_sb[:, kt, :], xt[:],
                                             start=st, stop=sp)
                            nc.tensor.matmul(vT_ps[:], wv_sb[:, kt, :], xt[:],
                                             start=st, stop=sp)
                        for h in range(HPC):
                            rope(q_ps[h], qT_sb[h][:, csl], cos_c, sin_c)
                        rope(k_ps, kT_sb[:, csl], cos_c, sin_c)
                        vts = pA.tile([128, QC], f16, tag="vts", bufs=2,
                                      name="vts")
                        nc.scalar.copy(vts[:], vT_ps[:])
                        for sb in range(NJ):
                            tr_ps = psA.tile([TB, TB], f16, tag="tr", bufs=1,
                                             name="tr_ps")
                            nc.tensor.transpose(tr_ps[:],
                                                vts[:, sb * TB:(sb + 1) * TB],
                                                ident_sb[:])
                            nc.scalar.copy(v_sb[:, c * NJ + sb, :], tr_ps[:])

                # wo reuses the wq slot; DMAs overlap phase B compute
                wo_sb = res.tile([128, NKT, FPC], f16, tag="bigw", name="wo_sb")
                for kt in range(NKT):
                    nc.sync.dma_start(out=wo_sb[:, kt, :],
                                      in_=wo_d[kt * 128:(kt + 1) * 128, :])

                # ---- phases B (attention) + chunked allgather + C, fused ----
                with tc.tile_pool(name="pB", bufs=1) as pB, \
                     tc.tile_pool(name="psB", bufs=1, space="PSUM") as psB, \
                     tc.tile_pool(name="pC", bufs=1) as pC, \
                     tc.tile_pool(name="psC", bufs=1, space="PSUM") as psC:

                    def phase_b(qcn):
                        qsl = slice(qcn * QC, (qcn + 1) * QC)
                        ntb = (qcn + 1) * NJ
                        ctx_ps = [psB.tile([128, QC], f32, tag=f"ctx{h}",
                                           bufs=1, name=f"ctx_ps{h}")
                                  for h in range(HPC)]
                        den_sb = [pB.tile([128, QC], f32, tag=f"den{h}",
                                          bufs=1, name=f"den_sb{h}")
                                  for h in range(HPC)]
                        den_r = [None] * HPC
                        for tb in range(ntb):
                            j = tb - qcn * NJ
                            p_list = []
                            for h in range(HPC):
                                s_ps = psB.tile([128, QC], f32, tag="s",
                                                bufs=2, name="s_ps")
                                nc.tensor.matmul(
                                    s_ps[:], kT_sb[:, tb * TB:(tb + 1) * TB],
                                    qT_sb[h][:, qsl], start=True, stop=True)
                                if j >= 0:
                                    nc.vector.tensor_add(s_ps[:], s_ps[:],
                                                         masks_sb[:, j, :])
                                p_sb = pB.tile([128, QC], f16, tag="p",
                                               bufs=8, name="p_sb")
                                nc.scalar.activation(p_sb[:], s_ps[:], EXP,
                                                     bias=ebias_sb[:],
                                                     scale=INV_SQRT_DH)
                                p_list.append(p_sb)
                                if tb == 0:
                                    nc.vector.tensor_copy(den_sb[h][:], p_sb[:])
                                elif tb == ntb - 1:
                                    dr = pB.tile([128, QC], f16, tag="denr_sb",
                                                 bufs=2, name="den_r")
                                    nc.vector.tensor_add(dr[:], den_sb[h][:],
                                                         p_sb[:])
                                    den_r[h] = dr
                                else:
                                    nc.vector.tensor_add(den_sb[h][:],
                                                         den_sb[h][:], p_sb[:])
                            for h in range(HPC):
                                nc.tensor.matmul(ctx_ps[h][:], v_sb[:, tb, :],
                                                 p_list[h][:],
                                                 start=(tb == 0),
                                                 stop=(tb == ntb - 1))
                        for h in range(HPC):
                            aux1 = psB.tile([128, QC], f32, tag="aux", bufs=1,
                                            name="aux1")
                            nc.tensor.matmul(aux1[:1, :], onesc_sb[:],
                                             den_r[h][:], start=True, stop=True)
                            recip_sb = pB.tile([1, QC], f16, tag="recip",
                                               bufs=2, name="recip_sb")
                            with nc.allow_low_precision(reason="softmax denom"):
                                nc.vector.reciprocal(recip_sb[:], aux1[:1, :])
                            aux2 = psB.tile([128, QC], f32, tag="aux", bufs=1,
                                            name="aux2")
                            nc.tensor.matmul(aux2[:], onesr_sb[:], recip_sb[:],
                                             start=True, stop=True)
                            ctmp = pB.tile([128, QC], f32, tag="ctmp",
                                           bufs=2, name="ctmp")
                            nc.scalar.copy(ctmp[:], ctx_ps[h][:])
                            ctx_sb = pB.tile([128, QC], f16, tag="ctxsb",
                                             bufs=4, name="ctx_sb")
                            nc.vector.tensor_mul(ctx_sb[:], ctmp[:], aux2[:])
                            nc.sync.dma_start(
                                out=ctx_loc[qcn][h * DH:(h + 1) * DH, :],
                                in_=ctx_sb[:])
                        nc.gpsimd.collective_compute(
                            "AllGather", mybir.AluOpType.bypass,
                            replica_groups=[list(range(N_CORES))],
                            ins=[ctx_loc[qcn].opt()],
                            outs=[ctx_all[qcn].opt()])

                    def phase_c(qcn):
                        for qb in range(NJ):
                            bsl = slice(qb * TB, (qb + 1) * TB)
                            o_ps = psC.tile([TB, FPC], f32, tag="o",
                                            bufs=1, name="o_ps")
                            for kt in range(NKT):
                                ct = pC.tile([128, TB], f16, tag="ct",
                                             bufs=6, name="ct")
                                nc.sync.dma_start(
                                    out=ct[:],
                                    in_=ctx_all[qcn][kt * 128:(kt + 1) * 128,
                                                     bsl])
                                nc.tensor.matmul(o_ps[:], ct[:],
                                                 wo_sb[:, kt, :],
                                                 start=(kt == 0),
                                                 stop=(kt == NKT - 1))
                            o_sb = pC.tile([TB, FPC], f32, tag="osb",
                                           bufs=3, name="o_sb")
                            nc.scalar.copy(o_sb[:], o_ps[:])
                            qrow = qcn * QC + qb * TB
                            nc.sync.dma_start(out=out_d[qrow:qrow + TB, :],
                                              in_=o_sb[:])

                    phase_b(0)
                    for qcn in range(1, NQC):
                        phase_b(qcn)
                        phase_c(qcn - 1)
                    phase_c(NQC - 1)
    nc.compile()
    return nc


def _host_consts():
    ident = np.eye(TB, dtype=np.float16)
    rotm = np.zeros((DH, DH), dtype=np.float16)
    half = DH // 2
    for d in range(half):
        rotm[d + half, d] = -1.0   # out[d] = -q[d+half]
        rotm[d, d + half] = 1.0    # out[d+half] = q[d]
    onesc = np.ones((TB, 1), dtype=np.float16)
    onesr = np.ones((1, DH), dtype=np.float16)
    maskb = np.zeros((NJ, TB, QC), dtype=np.float32)
    tloc = np.arange(TB)[:, None]
    qloc = np.arange(QC)[None, :]
    for j in range(NJ):
        maskb[j] = np.where(tloc + TB * j <= qloc, 0.0, NEG_BIAS)
    return ident, rotm, onesc, onesr, maskb


def kernel(x, mask, cos, sin, Wq, Wk, Wv, Wo):
    from concourse.bass_utils import run_bass_kernel_spmd

    if "nc" not in _CACHE:
        _CACHE["nc"] = _build_program()
    nc = _CACHE["nc"]

    x = np.asarray(x, dtype=np.float32)
    cos = np.asarray(cos, dtype=np.float32)
    sin = np.asarray(sin, dtype=np.float32)
    Wq = np.asarray(Wq, dtype=np.float32)
    Wk = np.asarray(Wk, dtype=np.float32)
    Wv = np.asarray(Wv, dtype=np.float32)
    Wo = np.asarray(Wo, dtype=np.float32)

    xT = np.ascontiguousarray(x[0].T).astype(np.float16)   # [D, S]
    cosT = np.ascontiguousarray(cos.T)                     # [DH, S]
    sinT = np.ascontiguousarray(sin.T)
    ident, rotm, onesc, onesr, maskb = _host_consts()

    in_maps = []
    for i in range(N_CORES):
        in_maps.append({
            "xT": xT,
            "wq": np.ascontiguousarray(Wq[:, i * FPC:(i + 1) * FPC]).astype(np.float16),
            "wk": np.ascontiguousarray(Wk[:, i * DH:(i + 1) * DH]).astype(np.float16),
            "wv": np.ascontiguousarray(Wv[:, i * DH:(i + 1) * DH]).astype(np.float16),
            "wo": np.ascontiguousarray(Wo[:, i * FPC:(i + 1) * FPC]).astype(np.float16),
            "cosT": cosT,
            "sinT": sinT,
            "maskb": maskb,
            "ident": ident,
            "rotm": rotm,
            "onesc": onesc,
            "onesr": onesr,
        })

    import os
    trace = bool(os.environ.get("BASS_TRACE"))
    res = run_bass_kernel_spmd(nc, in_maps, list(range(N_CORES)), trace=trace)
    _CACHE["last_exec_time_ns"] = res.exec_time_ns

    out = np.concatenate([res.results[i]["out"] for i in range(N_CORES)], axis=1)
    return out[None]

